# revision 31
# baseline (speedup 1.0000x reference)
"""DeFeat distillation loss on 8 Trainium2 NeuronCores (Bass/Tile).

Data-parallel over the batch dim (B=8 -> 1 batch element per core).

HBM traffic is the roofline, so the host downcasts features before
upload (the 2e-2 rel-err budget dwarfs the quantization noise): student
features, teacher features (bias folded: t'' = 16*(t-b)) and adaptation
weights all go as fp8e4m3 scaled by 16 (keeps them out of the subnormal
range; max|16*x| ~ 87 < 240).  The device computes 256*d in psum (the
subtract rescales t'' by another 16) and 65536*q in the accumulators;
the host divides back out.  DMA drops 46 MB -> 12 MB per core vs the
fp32 baseline.

Per core the student features stay channel-major [C=256, H*W]; the
teacher features are uploaded PIXEL-major, packed in 128-pixel groups
([128 px, group*256 + oc]).  Per 128-pixel group:
  psum[p, o] = sum_c s[c, p] * W[o, c]     [TensorE: stationary = s
               chunk, moving = W^T chunk; two kc-half matmuls]
so the adapted features land pixel-major.  Then per group (routes
chosen round-robin to balance engines):
  A: d = t' - psum (DVE), dd = Square(d) + accum_out (ACT)
  B: psum -= t' via a -I matmul (PE), dd = Square(psum) + accum (ACT)
  C: d = t' - psum (DVE), dd = d*d + row-reduce (DVE ttr)
The accum_out row-sum IS q_p = sum_c d^2 for the group's 128 pixels --
no column-sum matmuls, no 1-partition copies.  q columns collect in one
[128, 171] fp32 tile, DMA'd out once.  The host rasterizes the gt
masks, takes q, and finishes the masked sums + sqrt in float64.
"""

import os
import sys

for _p in ("/opt/trn_rl_repo", os.path.expanduser("~/.axon_site/_ro/trn_rl_repo")):
    if os.path.isdir(_p) and _p not in sys.path:
        sys.path.insert(0, _p)

import numpy as np
import ml_dtypes

BF16 = ml_dtypes.bfloat16
F8 = ml_dtypes.float8_e4m3
S_SCALE = 16.0                             # fp8 scale for s, W, and t'
Q_SCALE = 65536.0                          # q comes back scaled by 256^2

WEIGHT_GT = 0.004
WEIGHT_BG = 0.0002
STRIDES = (8, 16, 32, 64, 128)
SIZES = (128, 64, 32, 16, 8)
HWS = tuple(s * s for s in SIZES)          # (16384, 4096, 1024, 256, 64)
B, C, NBOX = 8, 256, 16
N_CORES = 8
N_LEVELS = 5
PL = (128, 128, 128, 128, 64)              # pixel rows per group (L4 only 64)
NG_L = (128, 32, 8, 2, 1)                  # 128-px groups per level
GOFF = (0, 128, 160, 168, 170)
NG = 171

# Blocks of pixel segments (lvl, p0, w). First blocks narrow so compute
# starts early; small levels fused mid-stream.
BLOCKS = [
    [(0, 0, 256)], [(0, 256, 768)], [(0, 1024, 1024)],
    [(0, 2048, 2048)], [(0, 4096, 2048)],
    [(1, 0, 2048)],
    [(0, 6144, 2048)], [(0, 8192, 2048)],
    [(2, 0, 1024), (3, 0, 256), (4, 0, 64)],
    [(0, 10240, 2048)], [(0, 12288, 2048)],
    [(1, 2048, 2048)],
    [(0, 14336, 2048)],
]

# Units are QUADS of four 128-px groups sharing one [128,1024] 2-bank
# psum (leftovers fall back to pairs/solos).  Quads alternate:
#   B-quad: PE -I-matmul subtract, 4 ACT squares straight from psum
#   D-quad: one merged DVE stt subtract [128,1024], 4 DVE squares
# Measured costs/group: ACT sq+accum .63us, DVE sq .35, DVE sub .33,
# PE negI .13 -> PE ~56, ACT ~57, DVE ~56.  (DVE squares can only read
# SBUF, so they pair with DVE subtracts; psum squares are ACT-only.)

WT_COLS = 5 * 2 * 256                      # fp8 W moving chunks


def _build_module():
    import concourse.mybir as mybir
    from concourse import bacc
    from concourse.tile import TileContext

    dt = mybir.dt
    nc = bacc.Bacc("TRN2", target_bir_lowering=False, debug=False,
                   num_devices=N_CORES)

    fs = [nc.dram_tensor(f"fs{l}", [C, HWS[l]], dt.float8e4, kind="ExternalInput")
          for l in range(N_LEVELS)]
    tt = [nc.dram_tensor(f"tt{l}", [PL[l], NG_L[l] * 256], dt.float8e4,
                         kind="ExternalInput")
          for l in range(N_LEVELS)]
    wt_d = nc.dram_tensor("wt", [128, WT_COLS], dt.float8e4, kind="ExternalInput")
    ni_d = nc.dram_tensor("ni", [128, 128], dt.float8e4, kind="ExternalInput")
    out_q = nc.dram_tensor("out_q", [128, NG], dt.float32, kind="ExternalOutput")

    SUB = mybir.AluOpType.subtract
    BYP = mybir.AluOpType.bypass
    MULT = mybir.AluOpType.mult
    ADD = mybir.AluOpType.add
    SQUARE = mybir.ActivationFunctionType.Square

    with TileContext(nc) as tc:
        with (
            tc.tile_pool(name="const", bufs=1) as const_pool,
            tc.tile_pool(name="feat", bufs=6) as feat_pool,
            tc.tile_pool(name="work", bufs=4) as work_pool,
            tc.tile_pool(name="ps", bufs=4, space="PSUM") as psum_pool,
        ):
            wt = const_pool.tile([128, WT_COLS], dt.float8e4)
            ni = const_pool.tile([128, 128], dt.float8e4)
            qcat = const_pool.tile([128, NG], dt.float32)
            nc.vector.memset(qcat[:], 0.0)

            # level-0 weights + negI first (small, unblocks first blocks)
            nc.sync.dma_start(out=wt[:, 0:512], in_=wt_d[:, 0:512])
            nc.sync.dma_start(out=ni[:], in_=ni_d[:])

            pair_ctr = 0
            first = True
            for segs in BLOCKS:
                W = sum(w for (_, _, w) in segs)
                NGRP = sum((w + 127) // 128 for (_, _, w) in segs)
                s_lo = feat_pool.tile([128, W], dt.float8e4, tag="s_lo")
                s_hi = feat_pool.tile([128, W], dt.float8e4, tag="s_hi")
                tT = feat_pool.tile([128, 256 * NGRP], dt.float8e4, tag="tT")
                bcol = 0
                goff = 0
                for (lvl, p0, w) in segs:
                    ngr = (w + 127) // 128
                    nc.sync.dma_start(out=s_lo[:, bcol:bcol + w],
                                      in_=fs[lvl][0:128, p0:p0 + w])
                    nc.sync.dma_start(out=s_hi[:, bcol:bcol + w],
                                      in_=fs[lvl][128:256, p0:p0 + w])
                    pl = PL[lvl]
                    g0 = (p0 // 128) * 256
                    nc.sync.dma_start(
                        out=tT[0:pl, 256 * goff:256 * (goff + ngr)],
                        in_=tt[lvl][0:pl, g0:g0 + 256 * ngr])
                    bcol += w
                    goff += ngr
                if first:
                    nc.sync.dma_start(out=wt[:, 512:WT_COLS],
                                      in_=wt_d[:, 512:WT_COLS])
                    first = False

                # 128-px groups: (lvl, scol, tcol, pw, q_idx)
                groups = []
                bcol = 0
                goff = 0
                for (lvl, p0, w) in segs:
                    ngr = (w + 127) // 128
                    for j in range(ngr):
                        pw = min(128, w - 128 * j)
                        groups.append((lvl, bcol + 128 * j, 256 * (goff + j),
                                       pw, GOFF[lvl] + p0 // 128 + j))
                    bcol += w
                    goff += ngr
                units = []
                i = 0
                while i < len(groups):
                    take = 1
                    while (take < 4 and i + take < len(groups)
                           and groups[i + take][3] == 128
                           and groups[i][3] == 128
                           and groups[i + take][2] ==
                           groups[i][2] + 256 * take):
                        take += 1
                    units.append(groups[i:i + take])
                    i += take

                for unit in units:
                    nu = len(unit)
                    # B needs all-128px groups: its -I matmul streams the
                    # full 128 tT rows (a 64-px group leaves rows 64:128
                    # uninitialized; 0 * NaN would poison the psum).
                    route = "B" if (pair_ctr % 2 == 0
                                    and unit[-1][3] == 128) else "D"
                    pair_ctr += 1
                    ps = psum_pool.tile([128, 1024], dt.float32, tag="ps")
                    for ui, (lvl, scol, tcol, pw, qg) in enumerate(unit):
                        o = 256 * ui
                        c0 = (lvl * 2) * 256
                        c1 = (lvl * 2 + 1) * 256
                        nc.tensor.matmul(ps[0:pw, o:o + 256],
                                         s_lo[:, scol:scol + pw],
                                         wt[:, c0:c0 + 256],
                                         start=True, stop=False)
                        nc.tensor.matmul(ps[0:pw, o:o + 256],
                                         s_hi[:, scol:scol + pw],
                                         wt[:, c1:c1 + 256],
                                         start=False, stop=(route != "B"))
                        if route == "B":
                            # psum -= t'' : stationary -16I, moving t'' chunk
                            nc.tensor.matmul(ps[0:pw, o:o + 256],
                                             ni[:, 0:pw],
                                             tT[0:128, tcol:tcol + 256],
                                             start=False, stop=True)

                    if route == "B":
                        for ui, (lvl, scol, tcol, pw, qg) in enumerate(unit):
                            o = 256 * ui
                            dd = work_pool.tile([128, 256], dt.bfloat16, tag="dd")
                            nc.scalar.activation(
                                dd[0:pw, :], ps[0:pw, o:o + 256], SQUARE,
                                accum_out=qcat[0:pw, qg:qg + 1])
                    else:
                        pw0 = unit[0][3]
                        tcol0 = unit[0][2]
                        wcols = 256 * nu
                        d = work_pool.tile([128, 1024], dt.bfloat16, tag="d")
                        # d = 16*t'' - psum  (rescales fp8 t'' to psum units)
                        nc.vector.scalar_tensor_tensor(
                            d[0:pw0, 0:wcols],
                            tT[0:pw0, tcol0:tcol0 + wcols], S_SCALE,
                            ps[0:pw0, 0:wcols],
                            op0=MULT, op1=SUB)
                        for ui, (lvl, scol, tcol, pw, qg) in enumerate(unit):
                            o = 256 * ui
                            dd = work_pool.tile([128, 256], dt.bfloat16, tag="dd")
                            nc.vector.scalar_tensor_tensor(
                                dd[0:pw, :],
                                d[0:pw, o:o + 256], 0.0,
                                d[0:pw, o:o + 256],
                                op0=BYP, op1=MULT,
                                accum_out=qcat[0:pw, qg:qg + 1])

            nc.sync.dma_start(out=out_q[:], in_=qcat[:])

    nc.compile()
    return nc


def _rasterize_masks(gt_bboxes):
    """Host-side mask rasterization, mirroring reference.gt_mask.

    Returns per-level [B, HW] float64 masks."""
    out = []
    for lvl in range(N_LEVELS):
        h = w = SIZES[lvl]
        stride = np.float32(STRIDES[lvl])
        q = np.floor(gt_bboxes.astype(np.float32) / stride).astype(np.int32)
        lx = np.minimum(q[..., 0], w - 1)
        ly = np.minimum(q[..., 1], h - 1)
        rx = np.minimum(q[..., 2], w - 1)
        ry = np.minimum(q[..., 3], h - 1)
        lm = np.zeros((B, h * w), np.float64)
        for b in range(B):
            m = np.zeros((h, w), bool)
            for i in range(gt_bboxes.shape[1]):
                if lx[b, i] == rx[b, i] or ly[b, i] == ry[b, i]:
                    m[ly[b, i], lx[b, i]] = True
                else:
                    m[ly[b, i]:ry[b, i], lx[b, i]:rx[b, i]] = True
            lm[b] = m.reshape(-1)
        out.append(lm)
    return out


_NC_CACHE = None


def _get_nc():
    global _NC_CACHE
    if _NC_CACHE is None:
        _NC_CACHE = _build_module()
    return _NC_CACHE


def _run(in_maps, trace=False, trace_cores=None):
    from concourse.bass_utils import run_bass_kernel_spmd

    kwargs = {}
    if trace:
        kwargs.update(trace=True, trace_cores=trace_cores or [0])
    return run_bass_kernel_spmd(_get_nc(), in_maps, core_ids=list(range(N_CORES)),
                                **kwargs)


def _pack_wt(inputs):
    """Moving-operand weight chunks, scaled by S_SCALE into fp8:
    wt[:, (l*2+k)*256 + o] = S_SCALE * W_l[o, k*128+c]."""
    wtp = np.zeros((128, WT_COLS), np.float32)
    for lvl in range(N_LEVELS):
        w = np.asarray(inputs[f"adapt_w{lvl}"], np.float32)
        for kc in range(2):
            wtp[:, (lvl * 2 + kc) * 256:(lvl * 2 + kc) * 256 + 256] = \
                w[:, kc * 128:(kc + 1) * 128].T
    return (wtp * S_SCALE).astype(F8)


def _prep_in_maps(inputs):
    wtp = _pack_wt(inputs)
    negi = (-S_SCALE * np.eye(128, dtype=np.float32)).astype(F8)
    in_maps = []
    for b in range(N_CORES):
        m = {"wt": wtp, "ni": negi}
        for lvl in range(N_LEVELS):
            hw = HWS[lvl]
            s = np.asarray(inputs[f"feat_s{lvl}"][b], np.float32).reshape(C, hw)
            m[f"fs{lvl}"] = np.ascontiguousarray((s * S_SCALE).astype(F8))
            bv = np.asarray(inputs[f"adapt_b{lvl}"], np.float32)
            t = np.asarray(inputs[f"feat_t{lvl}"][b], np.float32).reshape(C, hw)
            tp = (t - bv[:, None]) * S_SCALE
            g, pl = NG_L[lvl], PL[lvl]
            tpk = tp.reshape(C, g, pl).transpose(2, 1, 0).reshape(pl, g * 256)
            m[f"tt{lvl}"] = np.ascontiguousarray(tpk.astype(F8))
        in_maps.append(m)
    return in_maps


def kernel(_trace=False, _return_results=False, **inputs):
    gt_bboxes = np.asarray(inputs["gt_bboxes"], np.float32)
    masks = _rasterize_masks(gt_bboxes)
    in_maps = _prep_in_maps(inputs)

    res = _run(in_maps, trace=_trace)

    s_tot = np.zeros(N_LEVELS, np.float64)
    s_gt = np.zeros(N_LEVELS, np.float64)
    for c in range(N_CORES):
        q = res.results[c]["out_q"].astype(np.float64) / Q_SCALE
        for lvl in range(N_LEVELS):
            pl, g = PL[lvl], NG_L[lvl]
            qpix = q[0:pl, GOFF[lvl]:GOFF[lvl] + g].T.reshape(-1)
            mv = masks[lvl][c]
            s_tot[lvl] += qpix.sum()
            s_gt[lvl] += (qpix * mv).sum()

    loss = np.float64(0.0)
    for lvl in range(N_LEVELS):
        s_bg = s_tot[lvl] - s_gt[lvl]
        loss += WEIGHT_GT * np.sqrt(s_gt[lvl] + 1e-8) + \
            WEIGHT_BG * np.sqrt(s_bg + 1e-8)

    out = np.array(loss, dtype=np.float32)
    if _return_results:
        return out, res
    return out


# revision 32
# speedup vs baseline: 1.0325x; 1.0325x over previous
"""DeFeat distillation loss on 8 Trainium2 NeuronCores (Bass/Tile).

Data-parallel over the batch dim (B=8 -> 1 batch element per core).

HBM traffic is the roofline, so the host downcasts features before
upload (the 2e-2 rel-err budget dwarfs the quantization noise): student
features, teacher features (bias folded: t'' = 16*(t-b)) and adaptation
weights all go as fp8e4m3 scaled by 16 (keeps them out of the subnormal
range; max|16*x| ~ 87 < 240).  The device computes 256*d in psum (the
subtract rescales t'' by another 16) and 65536*q in the accumulators;
the host divides back out.  DMA drops 46 MB -> 12 MB per core vs the
fp32 baseline.

Per core the student features stay channel-major [C=256, H*W]; the
teacher features are uploaded PIXEL-major, packed in 128-pixel groups
([128 px, group*256 + oc]).  Per 128-pixel group:
  psum[p, o] = sum_c s[c, p] * W[o, c]     [TensorE: stationary = s
               chunk, moving = W^T chunk; two kc-half matmuls]
so the adapted features land pixel-major.  Then per group (routes
chosen round-robin to balance engines):
  A: d = t' - psum (DVE), dd = Square(d) + accum_out (ACT)
  B: psum -= t' via a -I matmul (PE), dd = Square(psum) + accum (ACT)
  C: d = t' - psum (DVE), dd = d*d + row-reduce (DVE ttr)
The accum_out row-sum IS q_p = sum_c d^2 for the group's 128 pixels --
no column-sum matmuls, no 1-partition copies.  q columns collect in one
[128, 171] fp32 tile, DMA'd out once.  The host rasterizes the gt
masks, takes q, and finishes the masked sums + sqrt in float64.
"""

import os
import sys

for _p in ("/opt/trn_rl_repo", os.path.expanduser("~/.axon_site/_ro/trn_rl_repo")):
    if os.path.isdir(_p) and _p not in sys.path:
        sys.path.insert(0, _p)

import numpy as np
import ml_dtypes

BF16 = ml_dtypes.bfloat16
F8 = ml_dtypes.float8_e4m3
S_SCALE = 16.0                             # fp8 scale for s, W, and t'
Q_SCALE = 65536.0                          # q comes back scaled by 256^2

WEIGHT_GT = 0.004
WEIGHT_BG = 0.0002
STRIDES = (8, 16, 32, 64, 128)
SIZES = (128, 64, 32, 16, 8)
HWS = tuple(s * s for s in SIZES)          # (16384, 4096, 1024, 256, 64)
B, C, NBOX = 8, 256, 16
N_CORES = 8
N_LEVELS = 5
PL = (128, 128, 128, 128, 64)              # pixel rows per group (L4 only 64)
NG_L = (128, 32, 8, 2, 1)                  # 128-px groups per level
GOFF = (0, 128, 160, 168, 170)
NG = 171

# Blocks of pixel segments (lvl, p0, w). First blocks narrow so compute
# starts early; small levels fused mid-stream.
BLOCKS = [
    [(0, 0, 512)], [(0, 512, 1024)],
    [(0, 1536, 2048)], [(0, 3584, 2048)],
    [(1, 0, 2048)],
    [(0, 5632, 2048)], [(0, 7680, 2048)],
    [(2, 0, 1024), (3, 0, 256), (4, 0, 64)],
    [(0, 9728, 2048)], [(0, 11776, 2048)],
    [(1, 2048, 2048)],
    [(0, 13824, 2560)],
]

# Units are QUADS of four 128-px groups sharing one [128,1024] 2-bank
# psum (leftovers fall back to pairs/solos).  Quads alternate:
#   B-quad: PE -I-matmul subtract, 4 ACT squares straight from psum
#   D-quad: one merged DVE stt subtract [128,1024], 4 DVE squares
# Measured costs/group: ACT sq+accum .63us, DVE sq .35, DVE sub .33,
# PE negI .13 -> PE ~56, ACT ~57, DVE ~56.  (DVE squares can only read
# SBUF, so they pair with DVE subtracts; psum squares are ACT-only.)

WT_COLS = 5 * 2 * 256                      # fp8 W moving chunks


def _build_module():
    import concourse.mybir as mybir
    from concourse import bacc
    from concourse.tile import TileContext

    dt = mybir.dt
    nc = bacc.Bacc("TRN2", target_bir_lowering=False, debug=False,
                   num_devices=N_CORES)

    fs = [nc.dram_tensor(f"fs{l}", [C, HWS[l]], dt.float8e4, kind="ExternalInput")
          for l in range(N_LEVELS)]
    tt = [nc.dram_tensor(f"tt{l}", [PL[l], NG_L[l] * 256], dt.float8e4,
                         kind="ExternalInput")
          for l in range(N_LEVELS)]
    wt_d = nc.dram_tensor("wt", [128, WT_COLS], dt.float8e4, kind="ExternalInput")
    ni_d = nc.dram_tensor("ni", [128, 128], dt.float8e4, kind="ExternalInput")
    out_q = nc.dram_tensor("out_q", [128, NG], dt.float32, kind="ExternalOutput")

    SUB = mybir.AluOpType.subtract
    BYP = mybir.AluOpType.bypass
    MULT = mybir.AluOpType.mult
    ADD = mybir.AluOpType.add
    SQUARE = mybir.ActivationFunctionType.Square

    with TileContext(nc) as tc:
        with (
            tc.tile_pool(name="const", bufs=1) as const_pool,
            tc.tile_pool(name="feat", bufs=6) as feat_pool,
            tc.tile_pool(name="work", bufs=4) as work_pool,
            tc.tile_pool(name="ps", bufs=4, space="PSUM") as psum_pool,
        ):
            wt = const_pool.tile([128, WT_COLS], dt.float8e4)
            ni = const_pool.tile([128, 128], dt.float8e4)
            qcat = const_pool.tile([128, NG], dt.float32)
            nc.vector.memset(qcat[:], 0.0)

            # level-0 weights + negI first (small, unblocks first blocks)
            nc.sync.dma_start(out=wt[:, 0:512], in_=wt_d[:, 0:512])
            nc.sync.dma_start(out=ni[:], in_=ni_d[:])

            pair_ctr = 0
            first = True
            for segs in BLOCKS:
                W = sum(w for (_, _, w) in segs)
                NGRP = sum((w + 127) // 128 for (_, _, w) in segs)
                s_lo = feat_pool.tile([128, W], dt.float8e4, tag="s_lo")
                s_hi = feat_pool.tile([128, W], dt.float8e4, tag="s_hi")
                tT = feat_pool.tile([128, 256 * NGRP], dt.float8e4, tag="tT")
                bcol = 0
                goff = 0
                for (lvl, p0, w) in segs:
                    ngr = (w + 127) // 128
                    nc.sync.dma_start(out=s_lo[:, bcol:bcol + w],
                                      in_=fs[lvl][0:128, p0:p0 + w])
                    nc.sync.dma_start(out=s_hi[:, bcol:bcol + w],
                                      in_=fs[lvl][128:256, p0:p0 + w])
                    pl = PL[lvl]
                    g0 = (p0 // 128) * 256
                    nc.sync.dma_start(
                        out=tT[0:pl, 256 * goff:256 * (goff + ngr)],
                        in_=tt[lvl][0:pl, g0:g0 + 256 * ngr])
                    bcol += w
                    goff += ngr
                if first:
                    nc.sync.dma_start(out=wt[:, 512:WT_COLS],
                                      in_=wt_d[:, 512:WT_COLS])
                    first = False

                # 128-px groups: (lvl, scol, tcol, pw, q_idx)
                groups = []
                bcol = 0
                goff = 0
                for (lvl, p0, w) in segs:
                    ngr = (w + 127) // 128
                    for j in range(ngr):
                        pw = min(128, w - 128 * j)
                        groups.append((lvl, bcol + 128 * j, 256 * (goff + j),
                                       pw, GOFF[lvl] + p0 // 128 + j))
                    bcol += w
                    goff += ngr
                units = []
                i = 0
                while i < len(groups):
                    take = 1
                    while (take < 4 and i + take < len(groups)
                           and groups[i + take][3] == 128
                           and groups[i][3] == 128
                           and groups[i + take][2] ==
                           groups[i][2] + 256 * take):
                        take += 1
                    units.append(groups[i:i + take])
                    i += take

                for unit in units:
                    nu = len(unit)
                    # B needs all-128px groups: its -I matmul streams the
                    # full 128 tT rows (a 64-px group leaves rows 64:128
                    # uninitialized; 0 * NaN would poison the psum).
                    route = "B" if (pair_ctr % 2 == 0
                                    and unit[-1][3] == 128) else "D"
                    pair_ctr += 1
                    ps = psum_pool.tile([128, 1024], dt.float32, tag="ps")
                    for ui, (lvl, scol, tcol, pw, qg) in enumerate(unit):
                        o = 256 * ui
                        c0 = (lvl * 2) * 256
                        c1 = (lvl * 2 + 1) * 256
                        nc.tensor.matmul(ps[0:pw, o:o + 256],
                                         s_lo[:, scol:scol + pw],
                                         wt[:, c0:c0 + 256],
                                         start=True, stop=False)
                        nc.tensor.matmul(ps[0:pw, o:o + 256],
                                         s_hi[:, scol:scol + pw],
                                         wt[:, c1:c1 + 256],
                                         start=False, stop=(route != "B"))
                        if route == "B":
                            # psum -= t'' : stationary -16I, moving t'' chunk
                            nc.tensor.matmul(ps[0:pw, o:o + 256],
                                             ni[:, 0:pw],
                                             tT[0:128, tcol:tcol + 256],
                                             start=False, stop=True)

                    if route == "B":
                        for ui, (lvl, scol, tcol, pw, qg) in enumerate(unit):
                            o = 256 * ui
                            dd = work_pool.tile([128, 256], dt.bfloat16, tag="dd")
                            nc.scalar.activation(
                                dd[0:pw, :], ps[0:pw, o:o + 256], SQUARE,
                                accum_out=qcat[0:pw, qg:qg + 1])
                    else:
                        pw0 = unit[0][3]
                        tcol0 = unit[0][2]
                        wcols = 256 * nu
                        d = work_pool.tile([128, 1024], dt.bfloat16, tag="d")
                        # d = 16*t'' - psum  (rescales fp8 t'' to psum units)
                        nc.vector.scalar_tensor_tensor(
                            d[0:pw0, 0:wcols],
                            tT[0:pw0, tcol0:tcol0 + wcols], S_SCALE,
                            ps[0:pw0, 0:wcols],
                            op0=MULT, op1=SUB)
                        for ui, (lvl, scol, tcol, pw, qg) in enumerate(unit):
                            o = 256 * ui
                            dd = work_pool.tile([128, 256], dt.bfloat16, tag="dd")
                            nc.vector.scalar_tensor_tensor(
                                dd[0:pw, :],
                                d[0:pw, o:o + 256], 0.0,
                                d[0:pw, o:o + 256],
                                op0=BYP, op1=MULT,
                                accum_out=qcat[0:pw, qg:qg + 1])

            nc.sync.dma_start(out=out_q[:], in_=qcat[:])

    nc.compile()
    return nc


def _rasterize_masks(gt_bboxes):
    """Host-side mask rasterization, mirroring reference.gt_mask.

    Returns per-level [B, HW] float64 masks."""
    out = []
    for lvl in range(N_LEVELS):
        h = w = SIZES[lvl]
        stride = np.float32(STRIDES[lvl])
        q = np.floor(gt_bboxes.astype(np.float32) / stride).astype(np.int32)
        lx = np.minimum(q[..., 0], w - 1)
        ly = np.minimum(q[..., 1], h - 1)
        rx = np.minimum(q[..., 2], w - 1)
        ry = np.minimum(q[..., 3], h - 1)
        lm = np.zeros((B, h * w), np.float64)
        for b in range(B):
            m = np.zeros((h, w), bool)
            for i in range(gt_bboxes.shape[1]):
                if lx[b, i] == rx[b, i] or ly[b, i] == ry[b, i]:
                    m[ly[b, i], lx[b, i]] = True
                else:
                    m[ly[b, i]:ry[b, i], lx[b, i]:rx[b, i]] = True
            lm[b] = m.reshape(-1)
        out.append(lm)
    return out


_NC_CACHE = None


def _get_nc():
    global _NC_CACHE
    if _NC_CACHE is None:
        _NC_CACHE = _build_module()
    return _NC_CACHE


def _run(in_maps, trace=False, trace_cores=None):
    from concourse.bass_utils import run_bass_kernel_spmd

    kwargs = {}
    if trace:
        kwargs.update(trace=True, trace_cores=trace_cores or [0])
    return run_bass_kernel_spmd(_get_nc(), in_maps, core_ids=list(range(N_CORES)),
                                **kwargs)


def _pack_wt(inputs):
    """Moving-operand weight chunks, scaled by S_SCALE into fp8:
    wt[:, (l*2+k)*256 + o] = S_SCALE * W_l[o, k*128+c]."""
    wtp = np.zeros((128, WT_COLS), np.float32)
    for lvl in range(N_LEVELS):
        w = np.asarray(inputs[f"adapt_w{lvl}"], np.float32)
        for kc in range(2):
            wtp[:, (lvl * 2 + kc) * 256:(lvl * 2 + kc) * 256 + 256] = \
                w[:, kc * 128:(kc + 1) * 128].T
    return (wtp * S_SCALE).astype(F8)


def _prep_in_maps(inputs):
    wtp = _pack_wt(inputs)
    negi = (-S_SCALE * np.eye(128, dtype=np.float32)).astype(F8)
    in_maps = []
    for b in range(N_CORES):
        m = {"wt": wtp, "ni": negi}
        for lvl in range(N_LEVELS):
            hw = HWS[lvl]
            s = np.asarray(inputs[f"feat_s{lvl}"][b], np.float32).reshape(C, hw)
            m[f"fs{lvl}"] = np.ascontiguousarray((s * S_SCALE).astype(F8))
            bv = np.asarray(inputs[f"adapt_b{lvl}"], np.float32)
            t = np.asarray(inputs[f"feat_t{lvl}"][b], np.float32).reshape(C, hw)
            tp = (t - bv[:, None]) * S_SCALE
            g, pl = NG_L[lvl], PL[lvl]
            tpk = tp.reshape(C, g, pl).transpose(2, 1, 0).reshape(pl, g * 256)
            m[f"tt{lvl}"] = np.ascontiguousarray(tpk.astype(F8))
        in_maps.append(m)
    return in_maps


def kernel(_trace=False, _return_results=False, **inputs):
    gt_bboxes = np.asarray(inputs["gt_bboxes"], np.float32)
    masks = _rasterize_masks(gt_bboxes)
    in_maps = _prep_in_maps(inputs)

    res = _run(in_maps, trace=_trace)

    s_tot = np.zeros(N_LEVELS, np.float64)
    s_gt = np.zeros(N_LEVELS, np.float64)
    for c in range(N_CORES):
        q = res.results[c]["out_q"].astype(np.float64) / Q_SCALE
        for lvl in range(N_LEVELS):
            pl, g = PL[lvl], NG_L[lvl]
            qpix = q[0:pl, GOFF[lvl]:GOFF[lvl] + g].T.reshape(-1)
            mv = masks[lvl][c]
            s_tot[lvl] += qpix.sum()
            s_gt[lvl] += (qpix * mv).sum()

    loss = np.float64(0.0)
    for lvl in range(N_LEVELS):
        s_bg = s_tot[lvl] - s_gt[lvl]
        loss += WEIGHT_GT * np.sqrt(s_gt[lvl] + 1e-8) + \
            WEIGHT_BG * np.sqrt(s_bg + 1e-8)

    out = np.array(loss, dtype=np.float32)
    if _return_results:
        return out, res
    return out


# revision 33
# speedup vs baseline: 1.0704x; 1.0367x over previous
"""DeFeat distillation loss on 8 Trainium2 NeuronCores (Bass/Tile).

Data-parallel over the batch dim (B=8 -> 1 batch element per core).

HBM traffic is the roofline, so the host downcasts features before
upload (the 2e-2 rel-err budget dwarfs the quantization noise): student
features, teacher features (bias folded: t'' = 16*(t-b)) and adaptation
weights all go as fp8e4m3 scaled by 16 (keeps them out of the subnormal
range; max|16*x| ~ 87 < 240).  The device computes 256*d in psum (the
subtract rescales t'' by another 16) and 65536*q in the accumulators;
the host divides back out.  DMA drops 46 MB -> 12 MB per core vs the
fp32 baseline.

Per core the student features stay channel-major [C=256, H*W]; the
teacher features are uploaded PIXEL-major, packed in 128-pixel groups
([128 px, group*256 + oc]).  Per 128-pixel group:
  psum[p, o] = sum_c s[c, p] * W[o, c]     [TensorE: stationary = s
               chunk, moving = W^T chunk; two kc-half matmuls]
so the adapted features land pixel-major.  Then per group (routes
chosen round-robin to balance engines):
  A: d = t' - psum (DVE), dd = Square(d) + accum_out (ACT)
  B: psum -= t' via a -I matmul (PE), dd = Square(psum) + accum (ACT)
  C: d = t' - psum (DVE), dd = d*d + row-reduce (DVE ttr)
The accum_out row-sum IS q_p = sum_c d^2 for the group's 128 pixels --
no column-sum matmuls, no 1-partition copies.  q columns collect in one
[128, 171] fp32 tile, DMA'd out once.  The host rasterizes the gt
masks, takes q, and finishes the masked sums + sqrt in float64.
"""

import os
import sys

for _p in ("/opt/trn_rl_repo", os.path.expanduser("~/.axon_site/_ro/trn_rl_repo")):
    if os.path.isdir(_p) and _p not in sys.path:
        sys.path.insert(0, _p)

import numpy as np
import ml_dtypes

BF16 = ml_dtypes.bfloat16
F8 = ml_dtypes.float8_e4m3
S_SCALE = 16.0                             # fp8 scale for s, W, and t'
Q_SCALE = 65536.0                          # q comes back scaled by 256^2

WEIGHT_GT = 0.004
WEIGHT_BG = 0.0002
STRIDES = (8, 16, 32, 64, 128)
SIZES = (128, 64, 32, 16, 8)
HWS = tuple(s * s for s in SIZES)          # (16384, 4096, 1024, 256, 64)
B, C, NBOX = 8, 256, 16
N_CORES = 8
N_LEVELS = 5
PL = (128, 128, 128, 128, 64)              # pixel rows per group (L4 only 64)
NG_L = (128, 32, 8, 2, 1)                  # 128-px groups per level
GOFF = (0, 128, 160, 168, 170)
NG = 171

# Blocks of pixel segments (lvl, p0, w). First blocks narrow so compute
# starts early; small levels fused mid-stream.
BLOCKS = [
    [(0, 0, 512)], [(0, 512, 1024)],
    [(0, 1536, 2048)], [(0, 3584, 2048)],
    [(1, 0, 2048)],
    [(0, 5632, 2048)], [(0, 7680, 2048)],
    [(2, 0, 1024), (3, 0, 256), (4, 0, 64)],
    [(0, 9728, 2048)], [(0, 11776, 2048)],
    [(1, 2048, 2048)],
    [(0, 13824, 2560)],
]

# Units are QUADS of four 128-px groups sharing one [128,1024] 2-bank
# psum (leftovers fall back to pairs/solos).  Quads alternate:
#   B-quad: PE -I-matmul subtract, 4 ACT squares straight from psum
#   D-quad: one merged DVE stt subtract [128,1024], 4 DVE squares
# Measured costs/group: ACT sq+accum .63us, DVE sq .35, DVE sub .33,
# PE negI .13 -> PE ~56, ACT ~57, DVE ~56.  (DVE squares can only read
# SBUF, so they pair with DVE subtracts; psum squares are ACT-only.)

WT_COLS = 5 * 2 * 256                      # fp8 W moving chunks


def _build_module():
    import concourse.mybir as mybir
    from concourse import bacc
    from concourse.tile import TileContext

    dt = mybir.dt
    nc = bacc.Bacc("TRN2", target_bir_lowering=False, debug=False,
                   num_devices=N_CORES)

    fs = [nc.dram_tensor(f"fs{l}", [C, HWS[l]], dt.float8e4, kind="ExternalInput")
          for l in range(N_LEVELS)]
    tt = [nc.dram_tensor(f"tt{l}", [PL[l], NG_L[l] * 256], dt.float8e4,
                         kind="ExternalInput")
          for l in range(N_LEVELS)]
    wt_d = nc.dram_tensor("wt", [128, WT_COLS], dt.float8e4, kind="ExternalInput")
    ni_d = nc.dram_tensor("ni", [128, 128], dt.float8e4, kind="ExternalInput")
    out_q = nc.dram_tensor("out_q", [128, NG], dt.float32, kind="ExternalOutput")

    SUB = mybir.AluOpType.subtract
    BYP = mybir.AluOpType.bypass
    MULT = mybir.AluOpType.mult
    ADD = mybir.AluOpType.add
    SQUARE = mybir.ActivationFunctionType.Square

    with TileContext(nc) as tc:
        with (
            tc.tile_pool(name="const", bufs=1) as const_pool,
            tc.tile_pool(name="feat", bufs=8) as feat_pool,
            tc.tile_pool(name="work", bufs=6) as work_pool,
            tc.tile_pool(name="ps", bufs=4, space="PSUM") as psum_pool,
        ):
            wt = const_pool.tile([128, WT_COLS], dt.float8e4)
            ni = const_pool.tile([128, 128], dt.float8e4)
            qcat = const_pool.tile([128, NG], dt.float32)
            nc.vector.memset(qcat[:], 0.0)

            # level-0 weights + negI first (small, unblocks first blocks)
            nc.sync.dma_start(out=wt[:, 0:512], in_=wt_d[:, 0:512])
            nc.sync.dma_start(out=ni[:], in_=ni_d[:])

            pair_ctr = 0
            first = True
            for segs in BLOCKS:
                W = sum(w for (_, _, w) in segs)
                NGRP = sum((w + 127) // 128 for (_, _, w) in segs)
                s_lo = feat_pool.tile([128, W], dt.float8e4, tag="s_lo")
                s_hi = feat_pool.tile([128, W], dt.float8e4, tag="s_hi")
                tT = feat_pool.tile([128, 256 * NGRP], dt.float8e4, tag="tT")
                bcol = 0
                goff = 0
                for (lvl, p0, w) in segs:
                    ngr = (w + 127) // 128
                    nc.sync.dma_start(out=s_lo[:, bcol:bcol + w],
                                      in_=fs[lvl][0:128, p0:p0 + w])
                    nc.sync.dma_start(out=s_hi[:, bcol:bcol + w],
                                      in_=fs[lvl][128:256, p0:p0 + w])
                    pl = PL[lvl]
                    g0 = (p0 // 128) * 256
                    nc.sync.dma_start(
                        out=tT[0:pl, 256 * goff:256 * (goff + ngr)],
                        in_=tt[lvl][0:pl, g0:g0 + 256 * ngr])
                    bcol += w
                    goff += ngr
                if first:
                    nc.sync.dma_start(out=wt[:, 512:WT_COLS],
                                      in_=wt_d[:, 512:WT_COLS])
                    first = False

                # 128-px groups: (lvl, scol, tcol, pw, q_idx)
                groups = []
                bcol = 0
                goff = 0
                for (lvl, p0, w) in segs:
                    ngr = (w + 127) // 128
                    for j in range(ngr):
                        pw = min(128, w - 128 * j)
                        groups.append((lvl, bcol + 128 * j, 256 * (goff + j),
                                       pw, GOFF[lvl] + p0 // 128 + j))
                    bcol += w
                    goff += ngr
                units = []
                i = 0
                while i < len(groups):
                    take = 1
                    while (take < 4 and i + take < len(groups)
                           and groups[i + take][3] == 128
                           and groups[i][3] == 128
                           and groups[i + take][2] ==
                           groups[i][2] + 256 * take):
                        take += 1
                    units.append(groups[i:i + take])
                    i += take

                for unit in units:
                    nu = len(unit)
                    # B needs all-128px groups: its -I matmul streams the
                    # full 128 tT rows (a 64-px group leaves rows 64:128
                    # uninitialized; 0 * NaN would poison the psum).
                    route = "B" if (pair_ctr % 2 == 0
                                    and unit[-1][3] == 128) else "D"
                    pair_ctr += 1
                    ps = psum_pool.tile([128, 1024], dt.float32, tag="ps")
                    for ui, (lvl, scol, tcol, pw, qg) in enumerate(unit):
                        o = 256 * ui
                        c0 = (lvl * 2) * 256
                        c1 = (lvl * 2 + 1) * 256
                        nc.tensor.matmul(ps[0:pw, o:o + 256],
                                         s_lo[:, scol:scol + pw],
                                         wt[:, c0:c0 + 256],
                                         start=True, stop=False)
                        nc.tensor.matmul(ps[0:pw, o:o + 256],
                                         s_hi[:, scol:scol + pw],
                                         wt[:, c1:c1 + 256],
                                         start=False, stop=(route != "B"))
                        if route == "B":
                            # psum -= t'' : stationary -16I, moving t'' chunk
                            nc.tensor.matmul(ps[0:pw, o:o + 256],
                                             ni[:, 0:pw],
                                             tT[0:128, tcol:tcol + 256],
                                             start=False, stop=True)

                    if route == "B":
                        for ui, (lvl, scol, tcol, pw, qg) in enumerate(unit):
                            o = 256 * ui
                            dd = work_pool.tile([128, 256], dt.bfloat16, tag="dd")
                            nc.scalar.activation(
                                dd[0:pw, :], ps[0:pw, o:o + 256], SQUARE,
                                accum_out=qcat[0:pw, qg:qg + 1])
                    else:
                        pw0 = unit[0][3]
                        tcol0 = unit[0][2]
                        wcols = 256 * nu
                        d = work_pool.tile([128, 1024], dt.bfloat16, tag="d")
                        # d = 16*t'' - psum  (rescales fp8 t'' to psum units)
                        nc.vector.scalar_tensor_tensor(
                            d[0:pw0, 0:wcols],
                            tT[0:pw0, tcol0:tcol0 + wcols], S_SCALE,
                            ps[0:pw0, 0:wcols],
                            op0=MULT, op1=SUB)
                        for ui, (lvl, scol, tcol, pw, qg) in enumerate(unit):
                            o = 256 * ui
                            dd = work_pool.tile([128, 256], dt.bfloat16, tag="dd")
                            nc.vector.scalar_tensor_tensor(
                                dd[0:pw, :],
                                d[0:pw, o:o + 256], 0.0,
                                d[0:pw, o:o + 256],
                                op0=BYP, op1=MULT,
                                accum_out=qcat[0:pw, qg:qg + 1])

            nc.sync.dma_start(out=out_q[:], in_=qcat[:])

    nc.compile()
    return nc


def _rasterize_masks(gt_bboxes):
    """Host-side mask rasterization, mirroring reference.gt_mask.

    Returns per-level [B, HW] float64 masks."""
    out = []
    for lvl in range(N_LEVELS):
        h = w = SIZES[lvl]
        stride = np.float32(STRIDES[lvl])
        q = np.floor(gt_bboxes.astype(np.float32) / stride).astype(np.int32)
        lx = np.minimum(q[..., 0], w - 1)
        ly = np.minimum(q[..., 1], h - 1)
        rx = np.minimum(q[..., 2], w - 1)
        ry = np.minimum(q[..., 3], h - 1)
        lm = np.zeros((B, h * w), np.float64)
        for b in range(B):
            m = np.zeros((h, w), bool)
            for i in range(gt_bboxes.shape[1]):
                if lx[b, i] == rx[b, i] or ly[b, i] == ry[b, i]:
                    m[ly[b, i], lx[b, i]] = True
                else:
                    m[ly[b, i]:ry[b, i], lx[b, i]:rx[b, i]] = True
            lm[b] = m.reshape(-1)
        out.append(lm)
    return out


_NC_CACHE = None


def _get_nc():
    global _NC_CACHE
    if _NC_CACHE is None:
        _NC_CACHE = _build_module()
    return _NC_CACHE


def _run(in_maps, trace=False, trace_cores=None):
    from concourse.bass_utils import run_bass_kernel_spmd

    kwargs = {}
    if trace:
        kwargs.update(trace=True, trace_cores=trace_cores or [0])
    return run_bass_kernel_spmd(_get_nc(), in_maps, core_ids=list(range(N_CORES)),
                                **kwargs)


def _pack_wt(inputs):
    """Moving-operand weight chunks, scaled by S_SCALE into fp8:
    wt[:, (l*2+k)*256 + o] = S_SCALE * W_l[o, k*128+c]."""
    wtp = np.zeros((128, WT_COLS), np.float32)
    for lvl in range(N_LEVELS):
        w = np.asarray(inputs[f"adapt_w{lvl}"], np.float32)
        for kc in range(2):
            wtp[:, (lvl * 2 + kc) * 256:(lvl * 2 + kc) * 256 + 256] = \
                w[:, kc * 128:(kc + 1) * 128].T
    return (wtp * S_SCALE).astype(F8)


def _prep_in_maps(inputs):
    wtp = _pack_wt(inputs)
    negi = (-S_SCALE * np.eye(128, dtype=np.float32)).astype(F8)
    in_maps = []
    for b in range(N_CORES):
        m = {"wt": wtp, "ni": negi}
        for lvl in range(N_LEVELS):
            hw = HWS[lvl]
            s = np.asarray(inputs[f"feat_s{lvl}"][b], np.float32).reshape(C, hw)
            m[f"fs{lvl}"] = np.ascontiguousarray((s * S_SCALE).astype(F8))
            bv = np.asarray(inputs[f"adapt_b{lvl}"], np.float32)
            t = np.asarray(inputs[f"feat_t{lvl}"][b], np.float32).reshape(C, hw)
            tp = (t - bv[:, None]) * S_SCALE
            g, pl = NG_L[lvl], PL[lvl]
            tpk = tp.reshape(C, g, pl).transpose(2, 1, 0).reshape(pl, g * 256)
            m[f"tt{lvl}"] = np.ascontiguousarray(tpk.astype(F8))
        in_maps.append(m)
    return in_maps


def kernel(_trace=False, _return_results=False, **inputs):
    gt_bboxes = np.asarray(inputs["gt_bboxes"], np.float32)
    masks = _rasterize_masks(gt_bboxes)
    in_maps = _prep_in_maps(inputs)

    res = _run(in_maps, trace=_trace)

    s_tot = np.zeros(N_LEVELS, np.float64)
    s_gt = np.zeros(N_LEVELS, np.float64)
    for c in range(N_CORES):
        q = res.results[c]["out_q"].astype(np.float64) / Q_SCALE
        for lvl in range(N_LEVELS):
            pl, g = PL[lvl], NG_L[lvl]
            qpix = q[0:pl, GOFF[lvl]:GOFF[lvl] + g].T.reshape(-1)
            mv = masks[lvl][c]
            s_tot[lvl] += qpix.sum()
            s_gt[lvl] += (qpix * mv).sum()

    loss = np.float64(0.0)
    for lvl in range(N_LEVELS):
        s_bg = s_tot[lvl] - s_gt[lvl]
        loss += WEIGHT_GT * np.sqrt(s_gt[lvl] + 1e-8) + \
            WEIGHT_BG * np.sqrt(s_bg + 1e-8)

    out = np.array(loss, dtype=np.float32)
    if _return_results:
        return out, res
    return out


# revision 34
# speedup vs baseline: 1.0747x; 1.0040x over previous
"""DeFeat distillation loss on 8 Trainium2 NeuronCores (Bass/Tile).

Data-parallel over the batch dim (B=8 -> 1 batch element per core).

HBM traffic is the roofline, so the host downcasts features before
upload (the 2e-2 rel-err budget dwarfs the quantization noise): student
features, teacher features (bias folded: t'' = 16*(t-b)) and adaptation
weights all go as fp8e4m3 scaled by 16 (keeps them out of the subnormal
range; max|16*x| ~ 87 < 240).  The device computes 256*d in psum (the
subtract rescales t'' by another 16) and 65536*q in the accumulators;
the host divides back out.  DMA drops 46 MB -> 12 MB per core vs the
fp32 baseline.

Per core the student features stay channel-major [C=256, H*W]; the
teacher features are uploaded PIXEL-major, packed in 128-pixel groups
([128 px, group*256 + oc]).  Per 128-pixel group:
  psum[p, o] = sum_c s[c, p] * W[o, c]     [TensorE: stationary = s
               chunk, moving = W^T chunk; two kc-half matmuls]
so the adapted features land pixel-major.  Then per group (routes
chosen round-robin to balance engines):
  A: d = t' - psum (DVE), dd = Square(d) + accum_out (ACT)
  B: psum -= t' via a -I matmul (PE), dd = Square(psum) + accum (ACT)
  C: d = t' - psum (DVE), dd = d*d + row-reduce (DVE ttr)
The accum_out row-sum IS q_p = sum_c d^2 for the group's 128 pixels --
no column-sum matmuls, no 1-partition copies.  q columns collect in one
[128, 171] fp32 tile, DMA'd out once.  The host rasterizes the gt
masks, takes q, and finishes the masked sums + sqrt in float64.
"""

import os
import sys

for _p in ("/opt/trn_rl_repo", os.path.expanduser("~/.axon_site/_ro/trn_rl_repo")):
    if os.path.isdir(_p) and _p not in sys.path:
        sys.path.insert(0, _p)

import numpy as np
import ml_dtypes

BF16 = ml_dtypes.bfloat16
F8 = ml_dtypes.float8_e4m3
S_SCALE = 16.0                             # fp8 scale for s, W, and t'
Q_SCALE = 65536.0                          # q comes back scaled by 256^2

WEIGHT_GT = 0.004
WEIGHT_BG = 0.0002
STRIDES = (8, 16, 32, 64, 128)
SIZES = (128, 64, 32, 16, 8)
HWS = tuple(s * s for s in SIZES)          # (16384, 4096, 1024, 256, 64)
B, C, NBOX = 8, 256, 16
N_CORES = 8
N_LEVELS = 5
PL = (128, 128, 128, 128, 64)              # pixel rows per group (L4 only 64)
NG_L = (128, 32, 8, 2, 1)                  # 128-px groups per level
GOFF = (0, 128, 160, 168, 170)
NG = 171

# Blocks of pixel segments (lvl, p0, w). First blocks narrow so compute
# starts early; small levels fused mid-stream.
BLOCKS = [
    [(0, 0, 512)], [(0, 512, 1024)],
    [(0, 1536, 2048)], [(0, 3584, 2048)],
    [(1, 0, 2048)],
    [(0, 5632, 2048)], [(0, 7680, 2048)],
    [(2, 0, 1024), (3, 0, 256), (4, 0, 64)],
    [(0, 9728, 2048)], [(0, 11776, 2048)],
    [(1, 2048, 2048)],
    [(0, 13824, 2560)],
]

# Units are QUADS of four 128-px groups sharing one [128,1024] 2-bank
# psum (leftovers fall back to pairs/solos).  Quads alternate:
#   B-quad: PE -I-matmul subtract, 4 ACT squares straight from psum
#   D-quad: one merged DVE stt subtract [128,1024], 4 DVE squares
# Measured costs/group: ACT sq+accum .63us, DVE sq .35, DVE sub .33,
# PE negI .13 -> PE ~56, ACT ~57, DVE ~56.  (DVE squares can only read
# SBUF, so they pair with DVE subtracts; psum squares are ACT-only.)

WT_COLS = 5 * 2 * 256                      # fp8 W moving chunks


def _build_module():
    import concourse.mybir as mybir
    from concourse import bacc
    from concourse.tile import TileContext

    dt = mybir.dt
    nc = bacc.Bacc("TRN2", target_bir_lowering=False, debug=False,
                   num_devices=N_CORES)

    fs = [nc.dram_tensor(f"fs{l}", [C, HWS[l]], dt.float8e4, kind="ExternalInput")
          for l in range(N_LEVELS)]
    tt = [nc.dram_tensor(f"tt{l}", [PL[l], NG_L[l] * 256], dt.float8e4,
                         kind="ExternalInput")
          for l in range(N_LEVELS)]
    wt_d = nc.dram_tensor("wt", [128, WT_COLS], dt.float8e4, kind="ExternalInput")
    ni_d = nc.dram_tensor("ni", [128, 128], dt.float8e4, kind="ExternalInput")
    out_q = nc.dram_tensor("out_q", [128, NG], dt.float32, kind="ExternalOutput")

    SUB = mybir.AluOpType.subtract
    BYP = mybir.AluOpType.bypass
    MULT = mybir.AluOpType.mult
    ADD = mybir.AluOpType.add
    SQUARE = mybir.ActivationFunctionType.Square

    with TileContext(nc) as tc:
        with (
            tc.tile_pool(name="const", bufs=1) as const_pool,
            tc.tile_pool(name="feat", bufs=12) as feat_pool,
            tc.tile_pool(name="work", bufs=8) as work_pool,
            tc.tile_pool(name="ps", bufs=4, space="PSUM") as psum_pool,
        ):
            wt = const_pool.tile([128, WT_COLS], dt.float8e4)
            ni = const_pool.tile([128, 128], dt.float8e4)
            qcat = const_pool.tile([128, NG], dt.float32)
            nc.vector.memset(qcat[:], 0.0)

            # level-0 weights + negI first (small, unblocks first blocks)
            nc.sync.dma_start(out=wt[:, 0:512], in_=wt_d[:, 0:512])
            nc.sync.dma_start(out=ni[:], in_=ni_d[:])

            pair_ctr = 0
            first = True
            for segs in BLOCKS:
                W = sum(w for (_, _, w) in segs)
                NGRP = sum((w + 127) // 128 for (_, _, w) in segs)
                s_lo = feat_pool.tile([128, W], dt.float8e4, tag="s_lo")
                s_hi = feat_pool.tile([128, W], dt.float8e4, tag="s_hi")
                tT = feat_pool.tile([128, 256 * NGRP], dt.float8e4, tag="tT")
                bcol = 0
                goff = 0
                for (lvl, p0, w) in segs:
                    ngr = (w + 127) // 128
                    nc.sync.dma_start(out=s_lo[:, bcol:bcol + w],
                                      in_=fs[lvl][0:128, p0:p0 + w])
                    nc.sync.dma_start(out=s_hi[:, bcol:bcol + w],
                                      in_=fs[lvl][128:256, p0:p0 + w])
                    pl = PL[lvl]
                    g0 = (p0 // 128) * 256
                    nc.sync.dma_start(
                        out=tT[0:pl, 256 * goff:256 * (goff + ngr)],
                        in_=tt[lvl][0:pl, g0:g0 + 256 * ngr])
                    bcol += w
                    goff += ngr
                if first:
                    nc.sync.dma_start(out=wt[:, 512:WT_COLS],
                                      in_=wt_d[:, 512:WT_COLS])
                    first = False

                # 128-px groups: (lvl, scol, tcol, pw, q_idx)
                groups = []
                bcol = 0
                goff = 0
                for (lvl, p0, w) in segs:
                    ngr = (w + 127) // 128
                    for j in range(ngr):
                        pw = min(128, w - 128 * j)
                        groups.append((lvl, bcol + 128 * j, 256 * (goff + j),
                                       pw, GOFF[lvl] + p0 // 128 + j))
                    bcol += w
                    goff += ngr
                units = []
                i = 0
                while i < len(groups):
                    take = 1
                    while (take < 4 and i + take < len(groups)
                           and groups[i + take][3] == 128
                           and groups[i][3] == 128
                           and groups[i + take][2] ==
                           groups[i][2] + 256 * take):
                        take += 1
                    units.append(groups[i:i + take])
                    i += take

                for unit in units:
                    nu = len(unit)
                    # B needs all-128px groups: its -I matmul streams the
                    # full 128 tT rows (a 64-px group leaves rows 64:128
                    # uninitialized; 0 * NaN would poison the psum).
                    route = "B" if (pair_ctr % 2 == 0
                                    and unit[-1][3] == 128) else "D"
                    pair_ctr += 1
                    ps = psum_pool.tile([128, 1024], dt.float32, tag="ps")
                    for ui, (lvl, scol, tcol, pw, qg) in enumerate(unit):
                        o = 256 * ui
                        c0 = (lvl * 2) * 256
                        c1 = (lvl * 2 + 1) * 256
                        nc.tensor.matmul(ps[0:pw, o:o + 256],
                                         s_lo[:, scol:scol + pw],
                                         wt[:, c0:c0 + 256],
                                         start=True, stop=False)
                        nc.tensor.matmul(ps[0:pw, o:o + 256],
                                         s_hi[:, scol:scol + pw],
                                         wt[:, c1:c1 + 256],
                                         start=False, stop=(route != "B"))
                        if route == "B":
                            # psum -= t'' : stationary -16I, moving t'' chunk
                            nc.tensor.matmul(ps[0:pw, o:o + 256],
                                             ni[:, 0:pw],
                                             tT[0:128, tcol:tcol + 256],
                                             start=False, stop=True)

                    if route == "B":
                        for ui, (lvl, scol, tcol, pw, qg) in enumerate(unit):
                            o = 256 * ui
                            dd = work_pool.tile([128, 256], dt.bfloat16, tag="dd")
                            nc.scalar.activation(
                                dd[0:pw, :], ps[0:pw, o:o + 256], SQUARE,
                                accum_out=qcat[0:pw, qg:qg + 1])
                    else:
                        pw0 = unit[0][3]
                        tcol0 = unit[0][2]
                        wcols = 256 * nu
                        d = work_pool.tile([128, 1024], dt.bfloat16, tag="d")
                        # d = 16*t'' - psum  (rescales fp8 t'' to psum units)
                        nc.vector.scalar_tensor_tensor(
                            d[0:pw0, 0:wcols],
                            tT[0:pw0, tcol0:tcol0 + wcols], S_SCALE,
                            ps[0:pw0, 0:wcols],
                            op0=MULT, op1=SUB)
                        for ui, (lvl, scol, tcol, pw, qg) in enumerate(unit):
                            o = 256 * ui
                            dd = work_pool.tile([128, 256], dt.bfloat16, tag="dd")
                            nc.vector.scalar_tensor_tensor(
                                dd[0:pw, :],
                                d[0:pw, o:o + 256], 0.0,
                                d[0:pw, o:o + 256],
                                op0=BYP, op1=MULT,
                                accum_out=qcat[0:pw, qg:qg + 1])

            nc.sync.dma_start(out=out_q[:], in_=qcat[:])

    nc.compile()
    return nc


def _rasterize_masks(gt_bboxes):
    """Host-side mask rasterization, mirroring reference.gt_mask.

    Returns per-level [B, HW] float64 masks."""
    out = []
    for lvl in range(N_LEVELS):
        h = w = SIZES[lvl]
        stride = np.float32(STRIDES[lvl])
        q = np.floor(gt_bboxes.astype(np.float32) / stride).astype(np.int32)
        lx = np.minimum(q[..., 0], w - 1)
        ly = np.minimum(q[..., 1], h - 1)
        rx = np.minimum(q[..., 2], w - 1)
        ry = np.minimum(q[..., 3], h - 1)
        lm = np.zeros((B, h * w), np.float64)
        for b in range(B):
            m = np.zeros((h, w), bool)
            for i in range(gt_bboxes.shape[1]):
                if lx[b, i] == rx[b, i] or ly[b, i] == ry[b, i]:
                    m[ly[b, i], lx[b, i]] = True
                else:
                    m[ly[b, i]:ry[b, i], lx[b, i]:rx[b, i]] = True
            lm[b] = m.reshape(-1)
        out.append(lm)
    return out


_NC_CACHE = None


def _get_nc():
    global _NC_CACHE
    if _NC_CACHE is None:
        _NC_CACHE = _build_module()
    return _NC_CACHE


def _run(in_maps, trace=False, trace_cores=None):
    from concourse.bass_utils import run_bass_kernel_spmd

    kwargs = {}
    if trace:
        kwargs.update(trace=True, trace_cores=trace_cores or [0])
    return run_bass_kernel_spmd(_get_nc(), in_maps, core_ids=list(range(N_CORES)),
                                **kwargs)


def _pack_wt(inputs):
    """Moving-operand weight chunks, scaled by S_SCALE into fp8:
    wt[:, (l*2+k)*256 + o] = S_SCALE * W_l[o, k*128+c]."""
    wtp = np.zeros((128, WT_COLS), np.float32)
    for lvl in range(N_LEVELS):
        w = np.asarray(inputs[f"adapt_w{lvl}"], np.float32)
        for kc in range(2):
            wtp[:, (lvl * 2 + kc) * 256:(lvl * 2 + kc) * 256 + 256] = \
                w[:, kc * 128:(kc + 1) * 128].T
    return (wtp * S_SCALE).astype(F8)


def _prep_in_maps(inputs):
    wtp = _pack_wt(inputs)
    negi = (-S_SCALE * np.eye(128, dtype=np.float32)).astype(F8)
    in_maps = []
    for b in range(N_CORES):
        m = {"wt": wtp, "ni": negi}
        for lvl in range(N_LEVELS):
            hw = HWS[lvl]
            s = np.asarray(inputs[f"feat_s{lvl}"][b], np.float32).reshape(C, hw)
            m[f"fs{lvl}"] = np.ascontiguousarray((s * S_SCALE).astype(F8))
            bv = np.asarray(inputs[f"adapt_b{lvl}"], np.float32)
            t = np.asarray(inputs[f"feat_t{lvl}"][b], np.float32).reshape(C, hw)
            tp = (t - bv[:, None]) * S_SCALE
            g, pl = NG_L[lvl], PL[lvl]
            tpk = tp.reshape(C, g, pl).transpose(2, 1, 0).reshape(pl, g * 256)
            m[f"tt{lvl}"] = np.ascontiguousarray(tpk.astype(F8))
        in_maps.append(m)
    return in_maps


def kernel(_trace=False, _return_results=False, **inputs):
    gt_bboxes = np.asarray(inputs["gt_bboxes"], np.float32)
    masks = _rasterize_masks(gt_bboxes)
    in_maps = _prep_in_maps(inputs)

    res = _run(in_maps, trace=_trace)

    s_tot = np.zeros(N_LEVELS, np.float64)
    s_gt = np.zeros(N_LEVELS, np.float64)
    for c in range(N_CORES):
        q = res.results[c]["out_q"].astype(np.float64) / Q_SCALE
        for lvl in range(N_LEVELS):
            pl, g = PL[lvl], NG_L[lvl]
            qpix = q[0:pl, GOFF[lvl]:GOFF[lvl] + g].T.reshape(-1)
            mv = masks[lvl][c]
            s_tot[lvl] += qpix.sum()
            s_gt[lvl] += (qpix * mv).sum()

    loss = np.float64(0.0)
    for lvl in range(N_LEVELS):
        s_bg = s_tot[lvl] - s_gt[lvl]
        loss += WEIGHT_GT * np.sqrt(s_gt[lvl] + 1e-8) + \
            WEIGHT_BG * np.sqrt(s_bg + 1e-8)

    out = np.array(loss, dtype=np.float32)
    if _return_results:
        return out, res
    return out
